# revision 60
# baseline (speedup 1.0000x reference)
"""Bass/Tile kernel for nn_BiDirectionalAddFFBlock on 8 TRN2 NeuronCores.

Sharding: core c -> (sample b = c//2, direction = c%2). Each core runs
LN + one mamba direction over one full sample (bwd cores receive the
host-flipped sample); a pair-wise ReduceScatter sums fwd+bwd and hands
each core half of its sample's tokens for the gelu/residual/FFN tail.

On-chip layout is feature-major ([d, l], d on partitions):
 - depthwise conv = 4 shifted scalar_tensor_tensor ops (per-partition taps)
 - selective scan = 16 per-state tensor_tensor_scan instructions per d-tile,
   decay exp(A[d,s]*dt) built on ACT with a per-partition scale AP
 - projections = PE matmuls (float32r for fp32 operands, bf16 elsewhere)
"""
import os
import sys

import numpy as np
import ml_dtypes

# concourse (Bass/Tile) normally arrives via the container's PYTHONPATH;
# append the known repo location as a fallback for bare environments.
if "/opt/trn_rl_repo" not in sys.path:
    sys.path.append("/opt/trn_rl_repo")

# A stale session on the axon terminal can leave the cores needing recovery,
# which turns the first execute of a fresh process into a multi-minute stall;
# an upfront core reset avoids that.
os.environ.setdefault("NEURON_RT_RESET_CORES", "1")

L = 2048          # sequence length per sample
D = 1024          # d_model
DI = 2048         # d_inner
DS = 16           # d_state
DTR = 64          # dt_rank
DCONV = 4
DFF = 4096
P = 128
NCORES = 8
LH = L // 2       # tokens per core in the FFN tail
NDT = DI // P     # 16 d-tiles
NHT = D // P      # 8 d_model tiles
NFT = DFF // P    # 32 dff tiles
NLC = 2           # scan L-chunks
LC = L // NLC     # 1024

# packed weight blob (bf16, shipped as per-core quarters, AllGathered):
# name -> (quarter_rows, cols); order defines flat offsets.
W_SPECS = [
    ("in_w", D // 4, 2 * DI),
    ("out_w", DI // 4, D),
    ("xproj", DI // 4, 96),
    ("dt_w", DTR // 4, DI),
    ("ff1", D // 4, DFF // 2),
    ("ff2", DFF // 8, D),
]
W_OFF = {}
_o = 0
for _n, _q, _c in W_SPECS:
    W_OFF[_n] = _o
    _o += _q * _c
WN = _o

# packed small-tensor blob (f32, shipped whole per core)
S_SPECS = [
    ("norm_g", (NHT, P)), ("norm_b", (NHT, P)), ("ffn_g", (NHT, P)),
    ("ffn_b", (NHT, P)), ("ff2_b", (NHT, P)), ("conv_w", (DI, DCONV)),
    ("conv_b", (NDT, P)), ("dt_b", (NDT, P)), ("Dp", (NDT, P)),
    ("ff1_b", (NFT // 2, P)), ("negA", (DI, DS)),
]
S_OFF = {}
_o = 0
for _n, _s in S_SPECS:
    S_OFF[_n] = _o
    _o += int(np.prod(_s))
SN = _o

_CACHE = {}


def _build(single=False):
    import concourse.bass as bass
    import concourse.mybir as mybir
    import concourse.tile as tile
    from concourse import bacc
    from concourse.masks import make_identity
    from contextlib import ExitStack

    dt = mybir.dt
    f32, f32r, bf16, fp16 = dt.float32, dt.float32r, dt.bfloat16, dt.float16
    AF = mybir.ActivationFunctionType
    OP = mybir.AluOpType
    AX = mybir.AxisListType

    nc = bacc.Bacc("TRN2", target_bir_lowering=False, debug=False,
                   enable_asserts=False, num_devices=(1 if single else NCORES))

    def inp(name, shape, dtype=f32):
        return nc.dram_tensor(name, shape, dtype, kind="ExternalInput").ap()

    # Replicated weights are shipped 1/4 each (row-block by sample index b),
    # packed into a single bf16 blob, and AllGathered on-device across the 4
    # cores of the same direction ([[0,2,4,6],[1,3,5,7]]); the FFN weights are
    # additionally split in half over d_ff by direction parity
    # (tensor-parallel FFN within each pair). All small f32 tensors ride in
    # one packed blob (sblob).
    xm = inp("xm", [L, D], bf16)              # mamba input (flipped on bwd cores)
    xhT = inp("xhT", [D, LH], bf16)           # unflipped token-half, transposed
    wblob = inp("wblob", [WN], bf16)          # packed weight quarters
    sblob = inp("sblob", [SN])                # packed small f32 tensors
    out = nc.dram_tensor("out", [D, LH], bf16, kind="ExternalOutput").ap()
    PARITY_GROUPS = [[0, 2, 4, 6], [1, 3, 5, 7]]
    PAIR_GROUPS = [[0, 1], [2, 3], [4, 5], [6, 7]]

    with tile.TileContext(nc) as tc, ExitStack() as top:
        # ---- DRAM scratch ----
        dram = top.enter_context(tc.tile_pool(name="dram", bufs=1, space="DRAM"))
        xs_dram = dram.tile([DI, L], bf16, name="xs_dram")
        z_dram = dram.tile([DI, L], bf16, name="z_dram")
        dt_dram = dram.tile([DI, L], fp16, name="dt_dram")
        u_dram = dram.tile([DI, L], bf16, name="u_dram")
        bc_dram = dram.tile([32, L], bf16, name="bc_dram")
        ar_in = dram.tile([2, D, LH], f32, name="ar_in")
        stats_dram = dram.tile([2, LH], f32, name="stats_dram")
        arh = dram.tile([D, LH], f32, name="arh")
        in_w_ag = dram.tile([D, 2 * DI], bf16, name="in_w_ag")
        xproj_ag = dram.tile([DI, 96], bf16, name="xproj_ag")
        dt_w_ag = dram.tile([DTR, DI], bf16, name="dt_w_ag")
        out_w_ag = dram.tile([DI, D], bf16, name="out_w_ag")
        ff1_ag = dram.tile([D, DFF // 2], bf16, name="ff1_ag")
        ff2_ag = dram.tile([DFF // 2, D], bf16, name="ff2_ag")
        hf_dram = dram.tile([D, LH], bf16, name="hf_dram")
        hf_all = dram.tile([2, D, LH], bf16, name="hf_all")
        ff_part = dram.tile([2, D, LH], f32, name="ff_part")
        ffh = dram.tile([D, LH], f32, name="ffh")

        # single AllGather of the packed weight blob (collectives cannot
        # source I/O tensors directly -> bounce via DRAM), then unpack into
        # 2-D weight buffers with dram-to-dram DMAs.
        wag = dram.tile([4, WN], bf16, name="wag")
        if single:
            for m in range(4):
                nc.sync.dma_start(wag[m, :], wblob)
        else:
            wbc = dram.tile([WN], bf16, name="wbc")
            nc.sync.dma_start(wbc[:], wblob)
            nc.gpsimd.collective_compute(
                "AllGather", OP.bypass, replica_groups=PARITY_GROUPS,
                ins=[wbc.opt()], outs=[wag.opt()])
        wq = {n: (q, c) for n, q, c in W_SPECS}
        for nm, dst in (("in_w", in_w_ag), ("out_w", out_w_ag),
                        ("xproj", xproj_ag), ("dt_w", dt_w_ag),
                        ("ff1", ff1_ag), ("ff2", ff2_ag)):
            off = W_OFF[nm]
            q, c = wq[nm]
            for m in range(4):
                nc.sync.dma_start(
                    dst[m * q:(m + 1) * q, :],
                    wag[m, off:off + q * c].rearrange("(r c) -> r c", c=c))

        # ---- small persistent SBUF ----
        persist = top.enter_context(tc.tile_pool(name="persist", bufs=1))
        bc_bf = persist.tile([32, L], bf16, name="bc_bf")       # B/C rows bf16
        carry = persist.tile([P, NDT * DS], f32, name="carry")
        zero1 = persist.tile([P, 1], f32, name="zero1")
        nc.vector.memset(zero1[:], 0.0)
        eps1 = persist.tile([P, 1], f32, name="eps1")
        nc.vector.memset(eps1[:], 1e-5)
        one1 = persist.tile([P, 1], f32, name="one1")
        nc.vector.memset(one1[:], 1.0)
        ident = persist.tile([P, P], f32, name="ident")
        make_identity(nc, ident)
        ident_bf = persist.tile([P, P], bf16, name="ident_bf")
        make_identity(nc, ident_bf)
        def sview(nm):
            off = S_OFF[nm]
            shp = dict(S_SPECS)[nm]
            return sblob[off:off + int(np.prod(shp))]

        negA_sb = persist.tile([P, NDT, DS], f32, name="negA_sb")
        nc.sync.dma_start(negA_sb[:],
                          sview("negA").rearrange("(t p s) -> p t s",
                                                  p=P, s=DS))
        convw_sb = persist.tile([P, NDT, DCONV], f32, name="convw_sb")
        nc.sync.dma_start(convw_sb[:],
                          sview("conv_w").rearrange("(t p k) -> p t k",
                                                    p=P, k=DCONV))
        convb_sb = persist.tile([P, NDT], f32, name="convb_sb")
        nc.sync.dma_start(convb_sb[:],
                          sview("conv_b").rearrange("(t p) -> p t", p=P))
        dtb_sb = persist.tile([P, NDT], f32, name="dtb_sb")
        nc.sync.dma_start(dtb_sb[:],
                          sview("dt_b").rearrange("(t p) -> p t", p=P))
        Dp_sb = persist.tile([P, NDT], f32, name="Dp_sb")
        nc.sync.dma_start(Dp_sb[:],
                          sview("Dp").rearrange("(t p) -> p t", p=P))
        ng_sb = persist.tile([P, NHT], f32, name="ng_sb")
        nc.sync.dma_start(ng_sb[:],
                          sview("norm_g").rearrange("(t p) -> p t", p=P))
        nb_sb = persist.tile([P, NHT], f32, name="nb_sb")
        nc.sync.dma_start(nb_sb[:],
                          sview("norm_b").rearrange("(t p) -> p t", p=P))
        fg_sb = persist.tile([P, NHT], f32, name="fg_sb")
        nc.sync.dma_start(fg_sb[:],
                          sview("ffn_g").rearrange("(t p) -> p t", p=P))
        fb_sb = persist.tile([P, NHT], f32, name="fb_sb")
        nc.sync.dma_start(fb_sb[:],
                          sview("ffn_b").rearrange("(t p) -> p t", p=P))
        f1b_sb = persist.tile([P, NFT // 2], f32, name="f1b_sb")
        nc.sync.dma_start(f1b_sb[:],
                          sview("ff1_b").rearrange("(t p) -> p t", p=P))
        f2b_sb = persist.tile([P, NHT], f32, name="f2b_sb")
        nc.sync.dma_start(f2b_sb[:],
                          sview("ff2_b").rearrange("(t p) -> p t", p=P))

        with tc.tile_pool(name="hTpool", bufs=1) as hTpool:
            hT_all = hTpool.tile([P, NHT, L], bf16, name="hT_all")
            dtrT = hTpool.tile([DTR, L], bf16, name="dtrT")

            # ============ Phase 0: LN(x) rowwise, transpose into hT ==========
            with tc.tile_pool(name="ph0", bufs=3) as ph0, \
                 tc.tile_pool(name="ph0ps", bufs=4, space="PSUM") as ph0ps:
                for lt in range(L // P):
                    xt = ph0.tile([P, D], bf16, name="xt")
                    nc.sync.dma_start(xt[:], xm[lt * P:(lt + 1) * P, :])
                    ssum = ph0.tile([P, 1], f32, name="ssum")
                    nc.vector.tensor_reduce(ssum[:], xt[:], AX.X, OP.add)
                    sq = ph0.tile([P, D], f32, name="sq")
                    sqsum = ph0.tile([P, 1], f32, name="sqsum")
                    nc.scalar.activation(sq[:], xt[:], AF.Square,
                                         accum_out=sqsum[:])
                    mu = ph0.tile([P, 1], f32, name="mu")
                    nc.scalar.mul(mu[:], ssum[:], 1.0 / D)
                    msq = ph0.tile([P, 1], f32, name="msq")
                    nc.scalar.mul(msq[:], sqsum[:], 1.0 / D)
                    musq = ph0.tile([P, 1], f32, name="musq")
                    nc.vector.tensor_tensor(musq[:], mu[:], mu[:], OP.mult)
                    var = ph0.tile([P, 1], f32, name="var")
                    nc.vector.tensor_tensor(var[:], msq[:], musq[:], OP.subtract)
                    std = ph0.tile([P, 1], f32, name="std")
                    nc.scalar.activation(std[:], var[:], AF.Sqrt, bias=eps1[:])
                    inv = ph0.tile([P, 1], f32, name="inv")
                    nc.vector.reciprocal(inv[:], std[:])
                    nmi = ph0.tile([P, 1], f32, name="nmi")
                    nc.vector.tensor_tensor(nmi[:], mu[:], inv[:], OP.mult)
                    nc.scalar.mul(nmi[:], nmi[:], -1.0)
                    hn = ph0.tile([P, D], bf16, name="hn")
                    nc.scalar.activation(hn[:], xt[:], AF.Identity,
                                         bias=nmi[:], scale=inv[:])
                    for dg in range(NHT // 4):
                        pst = ph0ps.tile([P, 4, P], bf16, name="pst")
                        for j in range(4):
                            dtl = dg * 4 + j
                            nc.tensor.transpose(
                                pst[:, j, :], hn[:, dtl * P:(dtl + 1) * P],
                                ident_bf[:])
                        for j in range(4):
                            dtl = dg * 4 + j
                            nc.vector.scalar_tensor_tensor(
                                hT_all[:, dtl, lt * P:(lt + 1) * P],
                                pst[:, j, :], ng_sb[:, dtl:dtl + 1],
                                nb_sb[:, dtl:dtl + 1].to_broadcast((P, P)),
                                OP.mult, OP.add)

            # ========= Phase 1: in_proj + conv + silu + xproj + z ============
            with tc.tile_pool(name="wpool", bufs=4) as wpool, \
                 tc.tile_pool(name="ph1", bufs=2) as ph1, \
                 tc.tile_pool(name="eps", bufs=1, space="PSUM") as epsp, \
                 tc.tile_pool(name="dblps", bufs=1, space="PSUM") as dblpsp:
                dbl_ps = dblpsp.tile([96, L], f32, name="dbl_ps")
                for et in range(32):
                    e_ps = epsp.tile([P, L], f32, name="e_ps")
                    wt = wpool.tile([P, NHT, P], bf16, name="wt", tag="wt")
                    nc.sync.dma_start(
                        wt[:], in_w_ag[:, et * P:(et + 1) * P]
                        .rearrange("(k p) c -> p k c", p=P))
                    for k in range(NHT):
                        for lq in range(4):
                            sl = slice(lq * 512, (lq + 1) * 512)
                            nc.tensor.matmul(
                                e_ps[:, sl], wt[:, k, :], hT_all[:, k, sl],
                                start=(k == 0), stop=(k == NHT - 1))
                    if et < NDT:
                        xsf = ph1.tile([P, L + 3], bf16, name="xsf")
                        nc.vector.memset(xsf[:, 0:3], 0.0)
                        nc.scalar.copy(xsf[:, 3:], e_ps[:])
                        parts = []
                        for k in range(DCONV):
                            pk = ph1.tile([P, L], bf16, name=f"cp{k}",
                                          tag=f"cp{k}")
                            nc.vector.tensor_scalar_mul(
                                pk[:], xsf[:, k:L + k], convw_sb[:, et, k:k + 1])
                            parts.append(pk)
                        pa = ph1.tile([P, L], bf16, name="pa", tag="pa")
                        nc.vector.tensor_tensor(pa[:], parts[0][:], parts[1][:],
                                                OP.add)
                        pb = ph1.tile([P, L], bf16, name="pb", tag="pb")
                        nc.vector.tensor_tensor(pb[:], parts[2][:], parts[3][:],
                                                OP.add)
                        cacc = ph1.tile([P, L], bf16, name="cacc")
                        nc.vector.tensor_tensor(cacc[:], pa[:], pb[:], OP.add)
                        xst = ph1.tile([P, L], bf16, name="xst")
                        nc.scalar.activation(xst[:], cacc[:], AF.Silu,
                                             bias=convb_sb[:, et:et + 1])
                        nc.sync.dma_start(xs_dram[et * P:(et + 1) * P, :], xst[:])
                        xw = wpool.tile([P, 96], bf16, name="xw", tag="xw")
                        nc.sync.dma_start(xw[:], xproj_ag[et * P:(et + 1) * P, :])
                        for lq in range(4):
                            sl = slice(lq * 512, (lq + 1) * 512)
                            nc.tensor.matmul(dbl_ps[:, sl], xw[:], xst[:, sl],
                                             start=(et == 0), stop=(et == NDT - 1))
                    else:
                        zs = ph1.tile([P, L], bf16, name="zs")
                        nc.scalar.activation(zs[:], e_ps[:], AF.Silu)
                        nc.sync.dma_start(
                            z_dram[(et - NDT) * P:(et - NDT + 1) * P, :], zs[:])
                nc.scalar.copy(dtrT[:], dbl_ps[0:DTR, :])
                nc.scalar.copy(bc_bf[:], dbl_ps[64:96, :])

            # =================== Phase 2: dt path ============================
            with tc.tile_pool(name="ph2", bufs=2) as ph2, \
                 tc.tile_pool(name="dtps", bufs=2, space="PSUM") as dtpsp:
                nc.sync.dma_start(bc_dram[:], bc_bf[:])
                dtw_sb = ph2.tile([DTR, DI], bf16, name="dtw_sb", bufs=1)
                nc.sync.dma_start(dtw_sb[:], dt_w_ag[:])
                for dti in range(NDT):
                    dt_ps = dtpsp.tile([P, L], f32, name="dt_ps")
                    for lq in range(4):
                        sl = slice(lq * 512, (lq + 1) * 512)
                        nc.tensor.matmul(
                            dt_ps[:, sl],
                            dtw_sb[:, dti * P:(dti + 1) * P], dtrT[:, sl],
                            start=True, stop=True)
                    spe = ph2.tile([P, L], f32, name="spe")
                    nc.scalar.activation(spe[:], dt_ps[:], AF.Exp,
                                         bias=dtb_sb[:, dti:dti + 1])
                    dtt = ph2.tile([P, L], fp16, name="dtt")
                    nc.scalar.activation(dtt[:], spe[:], AF.Ln, bias=one1[:])
                    nc.sync.dma_start(dt_dram[dti * P:(dti + 1) * P, :], dtt[:])
                    xsb = ph2.tile([P, L], bf16, name="xsb")
                    nc.sync.dma_start(xsb[:], xs_dram[dti * P:(dti + 1) * P, :])
                    ut = ph2.tile([P, L], bf16, name="ut")
                    nc.vector.tensor_tensor(ut[:], dtt[:], xsb[:], OP.mult)
                    nc.sync.dma_start(u_dram[dti * P:(dti + 1) * P, :], ut[:])

        # hT freed.  ============ Phase 3: selective scan ======================
        with tc.tile_pool(name="y2pool", bufs=1) as y2p:
            y2_all = y2p.tile([P, NDT, L], bf16, name="y2_all")
            with tc.tile_pool(name="bcastp", bufs=1) as bcp, \
                 tc.tile_pool(name="ph3s", bufs=2) as ph3s, \
                 tc.tile_pool(name="ph3t", bufs=2) as ph3t, \
                 tc.tile_pool(name="hcpool", bufs=3) as hcp, \
                 tc.tile_pool(name="treep", bufs=2) as treep:
                for lc in range(NLC):
                    lsl = slice(lc * LC, (lc + 1) * LC)
                    bcast = bcp.tile([P, 32, LC], bf16, name="bcast")
                    for j in range(32):
                        nc.sync.dma_start(
                            bcast[:, j, :],
                            bc_dram[j:j + 1, lsl].to_broadcast((P, LC)))
                    for dti in range(NDT):
                        dtt = ph3s.tile([P, LC], fp16, name="dtt3", tag="dtt3")
                        nc.sync.dma_start(dtt[:],
                                          dt_dram[dti * P:(dti + 1) * P, lsl])
                        ut = ph3s.tile([P, LC], bf16, name="ut3", tag="ut3")
                        nc.sync.dma_start(ut[:],
                                          u_dram[dti * P:(dti + 1) * P, lsl])
                        xsb = ph3s.tile([P, LC], bf16, name="xsb3", tag="xsb3")
                        nc.sync.dma_start(xsb[:],
                                          xs_dram[dti * P:(dti + 1) * P, lsl])
                        zt = ph3s.tile([P, LC], bf16, name="zt3", tag="zt3")
                        nc.sync.dma_start(zt[:],
                                          z_dram[dti * P:(dti + 1) * P, lsl])
                        SS = 4
                        ysum = None
                        for g in range(DS // SS):
                            s0 = g * SS
                            dtb = dtt[:].rearrange("p (s l) -> p s l", s=1) \
                                        .to_broadcast((P, SS, LC))
                            dAm = ph3t.tile([P, SS, LC], bf16, name="dAm",
                                            tag="dAm", bufs=1)
                            nc.vector.tensor_tensor(
                                dAm[:], dtb,
                                negA_sb[:, dti, s0:s0 + SS]
                                .to_broadcast((P, SS, LC)), OP.mult)
                            dAe = ph3t.tile([P, SS, LC], bf16, name="dAe",
                                            tag="dAe", bufs=1)
                            nc.scalar.activation(dAe[:], dAm[:], AF.Exp)
                            dBxm = ph3t.tile([P, SS, LC], bf16, name="dBxm",
                                             tag="dBxm", bufs=1)
                            nc.vector.tensor_tensor(
                                dBxm[:],
                                ut[:].rearrange("p (s l) -> p s l", s=1)
                                .to_broadcast((P, SS, LC)),
                                bcast[:, s0:s0 + SS, :], OP.mult)
                            hm = ph3t.tile([P, SS, LC], bf16, name="hm",
                                           tag="hm", bufs=1)
                            for si in range(SS):
                                cidx = dti * DS + s0 + si
                                nc.vector.tensor_tensor_scan(
                                    hm[:, si, :], dAe[:, si, :], dBxm[:, si, :],
                                    zero1[:] if lc == 0
                                    else carry[:, cidx:cidx + 1],
                                    OP.mult, OP.add)
                            if lc == 0 and NLC > 1:
                                base = dti * DS + s0
                                nc.scalar.copy(carry[:, base:base + SS],
                                               hm[:, :, LC - 1:])
                            hCm = ph3t.tile([P, SS, LC], bf16, name="hCm",
                                            tag="dAm", bufs=1)
                            nc.vector.tensor_tensor(
                                hCm[:], hm[:], bcast[:, DS + s0:DS + s0 + SS, :],
                                OP.mult)
                            t2 = treep.tile([P, 2, LC], bf16, name="t2",
                                            tag="t2", bufs=1)
                            nc.vector.tensor_tensor(t2[:], hCm[:, 0:2, :],
                                                    hCm[:, 2:4, :], OP.add)
                            yg = treep.tile([P, LC], f32, name="yg", tag="yg",
                                            bufs=2)
                            nc.vector.tensor_tensor(yg[:], t2[:, 0, :],
                                                    t2[:, 1, :], OP.add)
                            if ysum is None:
                                ysum = yg
                            else:
                                ynew = treep.tile([P, LC], f32, name=f"ys{g}",
                                                  tag=f"ys{g & 1}", bufs=1)
                                nc.vector.tensor_tensor(ynew[:], ysum[:],
                                                        yg[:], OP.add)
                                ysum = ynew
                        y = ysum             # f32 sum of all 16 states
                        y2a = ph3t.tile([P, LC], f32, name="y2a", tag="y2a",
                                        bufs=1)
                        nc.vector.scalar_tensor_tensor(
                            y2a[:], xsb[:], Dp_sb[:, dti:dti + 1], y[:],
                            OP.mult, OP.add)
                        nc.vector.tensor_tensor(y2_all[:, dti, lsl], y2a[:],
                                                zt[:], OP.mult)

            # ============ Phase 4: out_proj + ReduceScatter ==================
            with tc.tile_pool(name="ph4w", bufs=2) as ph4w, \
                 tc.tile_pool(name="ph4ps", bufs=2, space="PSUM") as ph4ps:
                for ot in range(NHT):
                    o_ps = ph4ps.tile([P, L], f32, name="o_ps")
                    wt = ph4w.tile([P, NDT, P], bf16, name="owt", tag="owt")
                    nc.sync.dma_start(
                        wt[:], out_w_ag[:, ot * P:(ot + 1) * P]
                        .rearrange("(k p) c -> p k c", p=P))
                    for k in range(NDT):
                        for lq in range(4):
                            sl = slice(lq * 512, (lq + 1) * 512)
                            nc.tensor.matmul(o_ps[:, sl], wt[:, k, :],
                                             y2_all[:, k, sl],
                                             start=(k == 0), stop=(k == NDT - 1))
                    o_sb = ph4w.tile([P, L], f32, name="o_sb", tag="o_sb",
                                     bufs=2)
                    nc.scalar.copy(o_sb[:], o_ps[:])
                    nc.sync.dma_start(ar_in[0, ot * P:(ot + 1) * P, :],
                                      o_sb[:, 0:LH])
                    nc.sync.dma_start(ar_in[1, ot * P:(ot + 1) * P, :],
                                      o_sb[:, LH:])
                if single:
                    nc.sync.dma_start(arh[:], ar_in[0])
                else:
                    nc.gpsimd.collective_compute(
                        "ReduceScatter", OP.add,
                        replica_groups=[[0, 1], [2, 3], [4, 5], [6, 7]],
                        ins=[ar_in.opt()], outs=[arh.opt()])

        # ====== Phase 5: gelu/residual + LN on my token half, then FFN =======
        # FFN is tensor-parallel over d_ff within each pair: AllGather the
        # LN'd activations to full L, each core matmuls its d_ff half, then
        # ReduceScatter the partial ff2 outputs back to token halves.
        NFH = NFT // 2
        with tc.tile_pool(name="x2pool", bufs=1) as x2p:
            x2T = x2p.tile([P, NHT, LH], f32, name="x2T")
            mub = x2p.tile([P, LH], f32, name="mub")
            invb = x2p.tile([P, LH], f32, name="invb")
            with tc.tile_pool(name="ph5a", bufs=2) as ph5, \
                 tc.tile_pool(name="statps", bufs=1, space="PSUM") as statps:
                musum_ps = statps.tile([1, LH], f32, name="musum_ps")
                sqsum_ps = statps.tile([1, LH], f32, name="sqsum_ps")
                onesv = ph5.tile([P, 1], f32, name="onesv", bufs=1)
                nc.vector.memset(onesv[:], 1.0)
                for dtl in range(NHT):
                    art = ph5.tile([P, LH], f32, name="art")
                    nc.sync.dma_start(art[:], arh[dtl * P:(dtl + 1) * P, :])
                    xh = ph5.tile([P, LH], bf16, name="xh")
                    nc.sync.dma_start(xh[:], xhT[dtl * P:(dtl + 1) * P, :])
                    nc.vector.tensor_tensor(art[:], art[:], xh[:], OP.add)
                    gl = ph5.tile([P, LH], f32, name="gl")
                    nc.scalar.activation(gl[:], art[:], AF.Gelu)
                    nc.vector.tensor_tensor(x2T[:, dtl, :], gl[:], xh[:], OP.add)
                    sq5 = ph5.tile([P, LH], f32, name="sq5")
                    nc.scalar.activation(sq5[:], x2T[:, dtl, :], AF.Square)
                    for lq in range(2):
                        sl = slice(lq * 512, (lq + 1) * 512)
                        nc.tensor.matmul(musum_ps[:, sl], onesv[:],
                                         x2T[:, dtl, sl],
                                         start=(dtl == 0), stop=(dtl == NHT - 1))
                        nc.tensor.matmul(sqsum_ps[:, sl], onesv[:],
                                         sq5[:, sl],
                                         start=(dtl == 0), stop=(dtl == NHT - 1))
                mu5 = ph5.tile([1, LH], f32, name="mu5", bufs=1)
                nc.scalar.mul(mu5[:], musum_ps[:], 1.0 / D)
                msq5 = ph5.tile([1, LH], f32, name="msq5", bufs=1)
                nc.scalar.mul(msq5[:], sqsum_ps[:], 1.0 / D)
                musq5 = ph5.tile([1, LH], f32, name="musq5", bufs=1)
                nc.vector.tensor_tensor(musq5[:], mu5[:], mu5[:], OP.mult)
                var5 = ph5.tile([1, LH], f32, name="var5", bufs=1)
                nc.vector.tensor_tensor(var5[:], msq5[:], musq5[:], OP.subtract)
                std5 = ph5.tile([1, LH], f32, name="std5", bufs=1)
                nc.scalar.activation(std5[:], var5[:], AF.Sqrt, bias=eps1[:1])
                inv5 = ph5.tile([1, LH], f32, name="inv5", bufs=1)
                nc.vector.reciprocal(inv5[:], std5[:])
                nc.sync.dma_start(stats_dram[0:1, :], mu5[:])
                nc.sync.dma_start(stats_dram[1:2, :], inv5[:])
                nc.sync.dma_start(mub[:],
                                  stats_dram[0:1, :].to_broadcast((P, LH)))
                nc.sync.dma_start(invb[:],
                                  stats_dram[1:2, :].to_broadcast((P, LH)))
            with tc.tile_pool(name="ph5b", bufs=3) as ph5b:
                for dtl in range(NHT):
                    t1 = ph5b.tile([P, LH], f32, name="t1")
                    nc.vector.tensor_tensor(t1[:], x2T[:, dtl, :], mub[:],
                                            OP.subtract)
                    nc.vector.tensor_tensor(t1[:], t1[:], invb[:], OP.mult)
                    hfm = ph5b.tile([P, LH], bf16, name="hfm")
                    nc.vector.scalar_tensor_tensor(
                        hfm[:], t1[:], fg_sb[:, dtl:dtl + 1],
                        fb_sb[:, dtl:dtl + 1].to_broadcast((P, LH)),
                        OP.mult, OP.add)
                    nc.sync.dma_start(hf_dram[dtl * P:(dtl + 1) * P, :], hfm[:])
            if single:
                nc.sync.dma_start(hf_all[0], hf_dram[:])
                nc.sync.dma_start(hf_all[1], hf_dram[:])
            else:
                nc.gpsimd.collective_compute(
                    "AllGather", OP.bypass, replica_groups=PAIR_GROUPS,
                    ins=[hf_dram.opt()], outs=[hf_all.opt()])
            with tc.tile_pool(name="hfpool", bufs=1) as hfp, \
                 tc.tile_pool(name="ffw", bufs=2) as ffw:
                hfT = hfp.tile([P, NHT, L], bf16, name="hfT")
                for m in range(2):
                    for dtl in range(NHT):
                        nc.sync.dma_start(hfT[:, dtl, m * LH:(m + 1) * LH],
                                          hf_all[m, dtl * P:(dtl + 1) * P, :])
                hf2 = hfp.tile([P, NFH, L], bf16, name="hf2")
                with tc.tile_pool(name="ff1ps", bufs=2, space="PSUM") as ff1ps:
                    for ft in range(NFH):
                        f_ps = ff1ps.tile([P, L], f32, name="f_ps")
                        wt = ffw.tile([P, NHT, P], bf16, name="fwt", tag="fwt")
                        nc.sync.dma_start(
                            wt[:], ff1_ag[:, ft * P:(ft + 1) * P]
                            .rearrange("(k p) c -> p k c", p=P))
                        for k in range(NHT):
                            for lq in range(4):
                                sl = slice(lq * 512, (lq + 1) * 512)
                                nc.tensor.matmul(f_ps[:, sl], wt[:, k, :],
                                                 hfT[:, k, sl],
                                                 start=(k == 0),
                                                 stop=(k == NHT - 1))
                        nc.scalar.activation(hf2[:, ft, :], f_ps[:], AF.Gelu,
                                             bias=f1b_sb[:, ft:ft + 1])
                with tc.tile_pool(name="ff2ps", bufs=2, space="PSUM") as ff2ps, \
                     tc.tile_pool(name="ph5c", bufs=2) as ph5c:
                    for ot in range(NHT):
                        o_ps = ff2ps.tile([P, L], f32, name="o_ps")
                        w2 = ffw.tile([P, NFH, P], bf16, name="f2wt",
                                      tag="f2wt")
                        nc.sync.dma_start(
                            w2[:], ff2_ag[:, ot * P:(ot + 1) * P]
                            .rearrange("(k p) c -> p k c", p=P))
                        for k in range(NFH):
                            for lq in range(4):
                                sl = slice(lq * 512, (lq + 1) * 512)
                                nc.tensor.matmul(o_ps[:, sl], w2[:, k, :],
                                                 hf2[:, k, sl],
                                                 start=(k == 0),
                                                 stop=(k == NFH - 1))
                        o_sb = ph5c.tile([P, L], f32, name="o_sb")
                        nc.scalar.copy(o_sb[:], o_ps[:])
                        nc.sync.dma_start(ff_part[0, ot * P:(ot + 1) * P, :],
                                          o_sb[:, 0:LH])
                        nc.sync.dma_start(ff_part[1, ot * P:(ot + 1) * P, :],
                                          o_sb[:, LH:])
                if single:
                    nc.sync.dma_start(ffh[:], ff_part[0])
                else:
                    nc.gpsimd.collective_compute(
                        "ReduceScatter", OP.add, replica_groups=PAIR_GROUPS,
                        ins=[ff_part.opt()], outs=[ffh.opt()])
                with tc.tile_pool(name="ph5d", bufs=2) as ph5d:
                    for ot in range(NHT):
                        fft = ph5d.tile([P, LH], f32, name="fft")
                        nc.sync.dma_start(fft[:], ffh[ot * P:(ot + 1) * P, :])
                        fin = ph5d.tile([P, LH], bf16, name="fin")
                        nc.vector.scalar_tensor_tensor(
                            fin[:], fft[:], f2b_sb[:, ot:ot + 1],
                            x2T[:, ot, :], OP.add, OP.add)
                        nc.sync.dma_start(out[ot * P:(ot + 1) * P, :], fin[:])

    nc.compile()
    return nc


def _get_nc():
    if "nc" not in _CACHE:
        _CACHE["nc"] = _build()
    return _CACHE["nc"]


def _prep_in_maps(inputs):
    bf = ml_dtypes.bfloat16
    f32 = np.float32
    p = {k: np.asarray(v) for k, v in inputs.items()}
    x = np.ascontiguousarray(p["x"], dtype=f32)          # [4, L, D]

    DFH = DFF // 2
    ff1_wT_full = p["ff1_w"].astype(f32).T.astype(bf)      # [D, DFF]
    ff2_wT_full = p["ff2_w"].astype(f32).T.astype(bf)      # [DFF, D]
    ff1_b_full = np.ascontiguousarray(p["ff1_b"], f32)     # [DFF]
    shared_s = [np.ascontiguousarray(p[k], f32).ravel()
                for k in ("norm_g", "norm_b", "ffn_g", "ffn_b", "ff2_b")]
    per_dir = {}
    for d, pre in ((0, "m1_"), (1, "m2_")):
        wfull = {
            "in_w": p[pre + "in_w"].astype(f32).T.astype(bf),
            "out_w": p[pre + "out_w"].astype(f32).T.astype(bf),
            "xproj": p[pre + "xproj_w"].astype(f32).T.astype(bf),
            "dt_w": p[pre + "dt_w"].astype(f32).T.astype(bf),
            "ff1": ff1_wT_full[:, d * DFH:(d + 1) * DFH],
            "ff2": ff2_wT_full[d * DFH:(d + 1) * DFH, :],
        }
        # order must match S_SPECS
        sb = np.concatenate(shared_s + [
            np.ascontiguousarray(p[pre + "conv_w"], f32).ravel(),
            np.ascontiguousarray(p[pre + "conv_b"], f32).ravel(),
            np.ascontiguousarray(p[pre + "dt_b"], f32).ravel(),
            np.ascontiguousarray(p[pre + "D"], f32).ravel(),
            ff1_b_full[d * DFH:(d + 1) * DFH],
            (-np.exp(p[pre + "Alog"].astype(f32))).ravel(),
        ])
        assert sb.size == SN
        per_dir[d] = (wfull, sb)
    in_maps = []
    for c in range(NCORES):
        b, d = c // 2, c % 2
        wfull, sb = per_dir[d]

        def q4(a, b=b):
            qq = a.shape[0] // 4
            return a[qq * b:qq * (b + 1)]

        wb = np.concatenate([np.ascontiguousarray(q4(wfull[n])).ravel()
                             for n, _, _ in W_SPECS])
        assert wb.size == WN and wb.dtype == bf
        xm_c = x[b] if d == 0 else x[b, ::-1]
        xh_c = np.ascontiguousarray(x[b, d * LH:(d + 1) * LH].T.astype(bf))
        in_maps.append({"xm": np.ascontiguousarray(xm_c.astype(bf)),
                        "xhT": xh_c, "wblob": wb, "sblob": sb})
    return in_maps


def _run(in_maps, **kwargs):
    from concourse import bass_utils
    nc = _get_nc()
    try:
        return bass_utils.run_bass_kernel_spmd(
            nc, in_maps, core_ids=list(range(NCORES)), **kwargs)
    except Exception:
        # One retry: the axon relay occasionally drops the worker mid-call.
        import time
        time.sleep(5)
        return bass_utils.run_bass_kernel_spmd(
            nc, in_maps, core_ids=list(range(NCORES)), **kwargs)


def kernel(**inputs):
    import threading

    box = {}

    def _warm():
        # jax/axon backend init and host-side input packing both overlap
        # with the Bass graph build on the main thread.
        try:
            import jax
            jax.devices()
        except Exception:
            pass
        box["im"] = _prep_in_maps(inputs)

    th = threading.Thread(target=_warm, daemon=True)
    th.start()
    _get_nc()
    th.join()
    res = _run(box["im"])
    x = np.asarray(inputs["x"])
    out = np.empty((4, L, D), np.float32)
    for c in range(NCORES):
        b, d = c // 2, c % 2
        out[b, d * LH:(d + 1) * LH] = res.results[c]["out"].T.astype(np.float32)
    return out.astype(x.dtype)


def time_on_device(inputs, iters=6):
    """Device-resident repeated-execute timing. Returns list of per-call
    seconds (first is warm-up/compile)."""
    import time
    import jax
    from jax.sharding import Mesh, PartitionSpec
    from jax.experimental.shard_map import shard_map
    import concourse.mybir as mybir
    from concourse import bass2jax
    from concourse.bass2jax import _bass_exec_p, install_neuronx_cc_hook, \
        partition_id_tensor

    install_neuronx_cc_hook()
    nc = _get_nc()
    in_maps = _prep_in_maps(inputs)
    n_cores = NCORES

    partition_name = (nc.partition_id_tensor.name
                      if nc.partition_id_tensor else None)
    in_names, out_names, out_avals, zero_outs = [], [], [], []
    for alloc in nc.m.functions[0].allocations:
        if not isinstance(alloc, mybir.MemoryLocationSet):
            continue
        name = alloc.memorylocations[0].name
        if alloc.kind == "ExternalInput":
            if name != partition_name:
                in_names.append(name)
        elif alloc.kind == "ExternalOutput":
            out_names.append(name)
            shape = tuple(alloc.tensor_shape)
            dtype = mybir.dt.np(alloc.dtype)
            out_avals.append(jax.core.ShapedArray(shape, dtype))
            zero_outs.append(np.zeros(shape, dtype))
    n_params = len(in_names)
    all_in_names = list(in_names) + list(out_names)
    if partition_name is not None:
        all_in_names.append(partition_name)

    def _body(*args):
        operands = list(args)
        if partition_name is not None:
            operands.append(partition_id_tensor())
        outs = _bass_exec_p.bind(
            *operands, out_avals=tuple(out_avals),
            in_names=tuple(all_in_names), out_names=tuple(out_names),
            lowering_input_output_aliases=(), sim_require_finite=True,
            sim_require_nnan=True, nc=nc)
        return tuple(outs)

    devices = jax.devices()[:n_cores]
    mesh = Mesh(np.asarray(devices), ("core",))
    n_outs = len(out_avals)
    in_specs = (PartitionSpec("core"),) * (n_params + n_outs)
    out_specs = (PartitionSpec("core"),) * n_outs
    fn = jax.jit(shard_map(_body, mesh=mesh, in_specs=in_specs,
                           out_specs=out_specs, check_rep=False),
                 keep_unused=True)
    concat_in = [np.concatenate([np.asarray(in_maps[c][nm])
                                 for c in range(n_cores)], axis=0)
                 for nm in in_names]
    concat_zeros = [np.zeros((n_cores * z.shape[0], *z.shape[1:]), z.dtype)
                    for z in zero_outs]
    from jax.sharding import NamedSharding
    shardings = [NamedSharding(mesh, PartitionSpec("core"))] * (n_params + n_outs)
    dev_args = [jax.device_put(a, s)
                for a, s in zip(concat_in + concat_zeros, shardings)]
    times = []
    for _ in range(iters):
        t0 = time.time()
        out = fn(*dev_args)
        jax.block_until_ready(out)
        times.append(time.time() - t0)
    return times

